# revision 5
# baseline (speedup 1.0000x reference)
import numpy as np
import jax
import jax.numpy as jnp
from jax import lax

# MemAE: B=512, M=2000, F=2304. Pure data parallel over 8 cores (64 samples
# each); memory bank + conv/BN params replicated; per-shard BN batch stats.
#
# Perf notes vs the plain fp32 version:
#  - convs/matmuls run with bf16 operands + fp32 accumulation
#    (preferred_element_type): TensorE is 4x faster on bf16 than fp32.
#  - the k=2 stride-2 transposed convs (d2, d3) are computed as a 1x1
#    einsum producing the 4 output parity phases + pixel interleave,
#    instead of lax lhs_dilation (which zero-stuffs the input and burns
#    4x the MACs on zeros).
N_CORES = 8
B = 512
BN_EPS = 1e-5
COS_EPS = 1e-8
SHRINK_EPS = 0.01

PARAM_NAMES = [
    'c1_w', 'c1_b', 'bn1_g', 'bn1_b', 'c2_w', 'c2_b', 'bn2_g', 'bn2_b',
    'c3_w', 'c3_b', 'bn3_g', 'bn3_b', 'c4_w', 'c4_b', 'bn4_g', 'bn4_b',
    'memory', 'd0_w', 'd0_b', 'dbn0_g', 'dbn0_b', 'd1_w', 'd1_b',
    'dbn1_g', 'dbn1_b', 'd2_w', 'd2_b', 'dbn2_g', 'dbn2_b', 'd3_w', 'd3_b',
]

_BF = jnp.float16  # fp16: same TensorE speed as bf16, 8x finer mantissa
_F32 = jnp.float32


def _conv(x, w, b, stride, pad):
    y = lax.conv_general_dilated(
        x.astype(_BF), w.astype(_BF), (stride, stride),
        [(pad, pad), (pad, pad)],
        dimension_numbers=('NCHW', 'OIHW', 'NCHW'),
        preferred_element_type=_F32)
    return y + b[None, :, None, None]


def _deconv_dilated(x, w, b, stride, pad, out_pad):
    k = w.shape[2]
    w2 = jnp.flip(w, (2, 3)).transpose(1, 0, 2, 3)
    p = [(k - 1 - pad, k - 1 - pad + out_pad)] * 2
    y = lax.conv_general_dilated(
        x.astype(_BF), w2.astype(_BF), (1, 1), p, lhs_dilation=(stride, stride),
        dimension_numbers=('NCHW', 'OIHW', 'NCHW'),
        preferred_element_type=_F32)
    return y + b[None, :, None, None]


def _deconv_k2s2(x, w, b, trim):
    # ConvTranspose2d with kernel 2, stride 2: every output pixel has exactly
    # one contributing tap, so out[n,o,2i+a,2j+b] = sum_c x[n,c,i,j] w[c,o,a,b].
    # trim=0 -> (pad=0, out_pad=0): full 2H x 2W output (d3).
    # trim=1 -> (pad=1, out_pad=1): drop first row/col of the 2H x 2W grid,
    #           keep 2H-1 (d2: 50 -> 49).
    n, c, h, wd = x.shape
    o = w.shape[1]
    t = jnp.einsum('ncij,coab->noiajb', x.astype(_BF), w.astype(_BF),
                   preferred_element_type=_F32)
    y = t.reshape(n, o, 2 * h, 2 * wd)
    if trim:
        y = y[:, :, trim:, trim:]
    return y + b[None, :, None, None]


def _bn_relu(x, g, b):
    # Cross-device batch stats (per-channel mean/E[x^2] pmean'd over the 8
    # shards) reproduce the reference's batch-512 BN exactly; per-shard
    # stats alone cost ~2e-2 absmax on the output.
    m = x.mean((0, 2, 3), keepdims=True)
    ms = (x * x).mean((0, 2, 3), keepdims=True)
    m = lax.pmean(m, axis_name='c')
    ms = lax.pmean(ms, axis_name='c')
    v = ms - m * m
    y = g[None, :, None, None] * (x - m) * lax.rsqrt(v + BN_EPS) + b[None, :, None, None]
    return jax.nn.relu(y)


def _forward(x, p):
    h = _bn_relu(_conv(x, p['c1_w'], p['c1_b'], 2, 1), p['bn1_g'], p['bn1_b'])
    h = _bn_relu(_conv(h, p['c2_w'], p['c2_b'], 2, 1), p['bn2_g'], p['bn2_b'])
    h = _bn_relu(_conv(h, p['c3_w'], p['c3_b'], 2, 1), p['bn3_g'], p['bn3_b'])
    h = _bn_relu(_conv(h, p['c4_w'], p['c4_b'], 2, 0), p['bn4_g'], p['bn4_b'])
    z = h.reshape(h.shape[0], -1)  # (b, 2304) fp32

    memory = p['memory']
    mem_bf = memory.astype(_BF)
    zn = jnp.linalg.norm(z, axis=1)
    mn = jnp.linalg.norm(memory, axis=1)
    sim = lax.dot_general(z.astype(_BF), mem_bf, (((1,), (1,)), ((), ())),
                          preferred_element_type=_F32)
    sim = sim / jnp.maximum(zn[:, None] * mn[None, :], COS_EPS)
    w = jax.nn.softmax(sim, axis=1)
    t = 1.0 / memory.shape[0]
    w = jax.nn.relu(w - t) * w / (jnp.abs(w - t) + SHRINK_EPS)
    w = w / jnp.sum(jnp.abs(w), axis=1, keepdims=True)
    z_hat = lax.dot_general(w.astype(_BF), mem_bf, (((1,), (0,)), ((), ())),
                            preferred_element_type=_F32)

    g = z_hat.reshape(-1, 64, 6, 6)
    g = _bn_relu(_deconv_dilated(g, p['d0_w'], p['d0_b'], 2, 0, 0), p['dbn0_g'], p['dbn0_b'])
    g = _bn_relu(_deconv_dilated(g, p['d1_w'], p['d1_b'], 2, 1, 0), p['dbn1_g'], p['dbn1_b'])
    g = _bn_relu(_deconv_k2s2(g, p['d2_w'], p['d2_b'], 1), p['dbn2_g'], p['dbn2_b'])
    g = jax.nn.sigmoid(_deconv_k2s2(g, p['d3_w'], p['d3_b'], 0))
    return g


_pmapped = None


def _get_pmapped():
    global _pmapped
    if _pmapped is None:
        _pmapped = jax.pmap(_forward, in_axes=(0, None), axis_name='c',
                            devices=jax.devices()[:N_CORES])
    return _pmapped


def kernel(**inputs):
    x = np.asarray(inputs['x'], np.float32)
    xs = x.reshape(N_CORES, B // N_CORES, *x.shape[1:])
    params = {k: jnp.asarray(np.asarray(inputs[k], np.float32)) for k in PARAM_NAMES}
    out = _get_pmapped()(jnp.asarray(xs), params)
    out = np.asarray(out)
    return out.reshape(B, *out.shape[2:])


# revision 8
# speedup vs baseline: 1.0637x; 1.0637x over previous
import numpy as np
import jax
import jax.numpy as jnp
from jax import lax

# MemAE: B=512, M=2000, F=2304. Pure data parallel over 8 cores (64 samples
# each); memory bank + conv/BN params replicated; per-shard BN batch stats.
#
# Perf notes vs the plain fp32 version:
#  - convs/matmuls run with bf16 operands + fp32 accumulation
#    (preferred_element_type): TensorE is 4x faster on bf16 than fp32.
#  - the k=2 stride-2 transposed convs (d2, d3) are computed as a 1x1
#    einsum producing the 4 output parity phases + pixel interleave,
#    instead of lax lhs_dilation (which zero-stuffs the input and burns
#    4x the MACs on zeros).
N_CORES = 8
B = 512
BN_EPS = 1e-5
COS_EPS = 1e-8
SHRINK_EPS = 0.01

PARAM_NAMES = [
    'c1_w', 'c1_b', 'bn1_g', 'bn1_b', 'c2_w', 'c2_b', 'bn2_g', 'bn2_b',
    'c3_w', 'c3_b', 'bn3_g', 'bn3_b', 'c4_w', 'c4_b', 'bn4_g', 'bn4_b',
    'memory', 'd0_w', 'd0_b', 'dbn0_g', 'dbn0_b', 'd1_w', 'd1_b',
    'dbn1_g', 'dbn1_b', 'd2_w', 'd2_b', 'dbn2_g', 'dbn2_b', 'd3_w', 'd3_b',
]

_BF = jnp.float16  # fp16: same TensorE speed as bf16, 8x finer mantissa
_F32 = jnp.float32


def _conv(x, w, b, stride, pad):
    y = lax.conv_general_dilated(
        x.astype(_BF), w.astype(_BF), (stride, stride),
        [(pad, pad), (pad, pad)],
        dimension_numbers=('NCHW', 'OIHW', 'NCHW'),
        preferred_element_type=_F32)
    return y + b[None, :, None, None]


def _deconv_dilated(x, w, b, stride, pad, out_pad):
    k = w.shape[2]
    w2 = jnp.flip(w, (2, 3)).transpose(1, 0, 2, 3)
    p = [(k - 1 - pad, k - 1 - pad + out_pad)] * 2
    y = lax.conv_general_dilated(
        x.astype(_BF), w2.astype(_BF), (1, 1), p, lhs_dilation=(stride, stride),
        dimension_numbers=('NCHW', 'OIHW', 'NCHW'),
        preferred_element_type=_F32)
    return y + b[None, :, None, None]


def _conv_raw(x, w, pad_hw):
    return lax.conv_general_dilated(
        x.astype(_BF), w.astype(_BF), (1, 1), pad_hw,
        dimension_numbers=('NCHW', 'OIHW', 'NCHW'),
        preferred_element_type=_F32)


def _deconv_k3s2(x, w, b, pad):
    # ConvTranspose2d k=3, s=2 via output-parity decomposition: each parity
    # class is a tiny dense conv on the un-dilated input (no zero-stuffing).
    # pad=0 (d0): H -> 2H+1. even outputs: taps (w[2]@x[i-1], w[0]@x[i]) i.e.
    #   2-tap conv with pad (1,1); odd outputs: w[1]@x[i].
    # pad=1 (d1): H -> 2H-1. even: w[1]@x[i]; odd: (w[2]@x[i], w[0]@x[i+1])
    #   i.e. 2-tap conv, no pad.
    # In both cases the even grid is one larger than the odd grid and
    # out = 2*he - 1 with out[2i+a] from parity class a.
    wt = w.transpose(1, 0, 2, 3)  # torch ConvT (in,out,kh,kw) -> (out,in,..)
    if pad == 0:
        t_e, p_e = [2, 0], (1, 1)
        t_o, p_o = [1], (0, 0)
    else:
        t_e, p_e = [1], (0, 0)
        t_o, p_o = [2, 0], (0, 0)

    def sub(ty, tx, py, px):
        return _conv_raw(x, wt[:, :, ty][:, :, :, tx], [py, px])

    ee = sub(t_e, t_e, p_e, p_e)
    eo = sub(t_e, t_o, p_e, p_o)
    oe = sub(t_o, t_e, p_o, p_e)
    oo = sub(t_o, t_o, p_o, p_o)
    n, c, he, we = ee.shape

    def ilv_cols(a, bsmall):
        bp = jnp.pad(bsmall, ((0, 0), (0, 0), (0, 0), (0, 1)))
        s = jnp.stack([a, bp], axis=-1).reshape(n, c, a.shape[2], 2 * we)
        return s[..., :2 * we - 1]

    row_e = ilv_cols(ee, eo)            # [n,c,he,2we-1]
    row_o = ilv_cols(oe, oo)            # [n,c,he-1,2we-1]
    row_o = jnp.pad(row_o, ((0, 0), (0, 0), (0, 1), (0, 0)))
    out = jnp.stack([row_e, row_o], axis=3).reshape(n, c, 2 * he, 2 * we - 1)
    out = out[:, :, :2 * he - 1, :]
    return out + b[None, :, None, None]


def _deconv_k2s2(x, w, b, trim):
    # ConvTranspose2d with kernel 2, stride 2: every output pixel has exactly
    # one contributing tap, so out[n,o,2i+a,2j+b] = sum_c x[n,c,i,j] w[c,o,a,b].
    # trim=0 -> (pad=0, out_pad=0): full 2H x 2W output (d3).
    # trim=1 -> (pad=1, out_pad=1): drop first row/col of the 2H x 2W grid,
    #           keep 2H-1 (d2: 50 -> 49).
    n, c, h, wd = x.shape
    o = w.shape[1]
    t = jnp.einsum('ncij,coab->noiajb', x.astype(_BF), w.astype(_BF),
                   preferred_element_type=_F32)
    y = t.reshape(n, o, 2 * h, 2 * wd)
    if trim:
        y = y[:, :, trim:, trim:]
    return y + b[None, :, None, None]


def _bn_relu(x, g, b):
    # Cross-device batch stats (per-channel mean/E[x^2] pmean'd over the 8
    # shards) reproduce the reference's batch-512 BN exactly; per-shard
    # stats alone cost ~2e-2 absmax on the output.
    m = x.mean((0, 2, 3), keepdims=True)
    ms = (x * x).mean((0, 2, 3), keepdims=True)
    m = lax.pmean(m, axis_name='c')
    ms = lax.pmean(ms, axis_name='c')
    v = ms - m * m
    y = g[None, :, None, None] * (x - m) * lax.rsqrt(v + BN_EPS) + b[None, :, None, None]
    return jax.nn.relu(y)


def _forward(x, p):
    h = _bn_relu(_conv(x, p['c1_w'], p['c1_b'], 2, 1), p['bn1_g'], p['bn1_b'])
    h = _bn_relu(_conv(h, p['c2_w'], p['c2_b'], 2, 1), p['bn2_g'], p['bn2_b'])
    h = _bn_relu(_conv(h, p['c3_w'], p['c3_b'], 2, 1), p['bn3_g'], p['bn3_b'])
    h = _bn_relu(_conv(h, p['c4_w'], p['c4_b'], 2, 0), p['bn4_g'], p['bn4_b'])
    z = h.reshape(h.shape[0], -1)  # (b, 2304) fp32

    memory = p['memory']
    mem_bf = memory.astype(_BF)
    zn = jnp.linalg.norm(z, axis=1)
    mn = jnp.linalg.norm(memory, axis=1)
    sim = lax.dot_general(z.astype(_BF), mem_bf, (((1,), (1,)), ((), ())),
                          preferred_element_type=_F32)
    sim = sim / jnp.maximum(zn[:, None] * mn[None, :], COS_EPS)
    w = jax.nn.softmax(sim, axis=1)
    t = 1.0 / memory.shape[0]
    w = jax.nn.relu(w - t) * w / (jnp.abs(w - t) + SHRINK_EPS)
    w = w / jnp.sum(jnp.abs(w), axis=1, keepdims=True)
    z_hat = lax.dot_general(w.astype(_BF), mem_bf, (((1,), (0,)), ((), ())),
                            preferred_element_type=_F32)

    g = z_hat.reshape(-1, 64, 6, 6)
    g = _bn_relu(_deconv_k3s2(g, p['d0_w'], p['d0_b'], 0), p['dbn0_g'], p['dbn0_b'])
    g = _bn_relu(_deconv_k3s2(g, p['d1_w'], p['d1_b'], 1), p['dbn1_g'], p['dbn1_b'])
    g = _bn_relu(_deconv_k2s2(g, p['d2_w'], p['d2_b'], 1), p['dbn2_g'], p['dbn2_b'])
    g = jax.nn.sigmoid(_deconv_k2s2(g, p['d3_w'], p['d3_b'], 0))
    return g


_pmapped = None


def _get_pmapped():
    global _pmapped
    if _pmapped is None:
        _pmapped = jax.pmap(_forward, in_axes=(0, None), axis_name='c',
                            devices=jax.devices()[:N_CORES])
    return _pmapped


def kernel(**inputs):
    x = np.asarray(inputs['x'], np.float32)
    xs = x.reshape(N_CORES, B // N_CORES, *x.shape[1:])
    params = {k: jnp.asarray(np.asarray(inputs[k], np.float32)) for k in PARAM_NAMES}
    out = _get_pmapped()(jnp.asarray(xs), params)
    out = np.asarray(out)
    return out.reshape(B, *out.shape[2:])
